# revision 1
# baseline (speedup 1.0000x reference)
"""ExpertScatter TRN2 kernel.

reference semantics:
    X = einsum('bekj,eji->beki', Y, W)          # per-head projection
    out[b] = zeros([T, I]); out[b, Ind[b,e,k]] += X[b,e,k]

Strategy (data-parallel over batch, 1 batch per NeuronCore):
  Phase A: per head e, matmul X_chunk[128 rows, 1024] = Yt_chunk.T @ W[e]
           (fp16 operands by default; float32r available = full PE rate
           with fp32 data), write X to an internal HBM staging buffer in
           natural row order (fp16 halves the round-trip traffic).
  Host precomputes a global sort of the 16384 rows of each batch by target
  slot, padded to a fixed PT rows per 128-slot output tile.
  Phase B: per output tile (128 slots), dma_gather the contributing rows
           (PT of them) into SBUF, build one-hot selection matrices on DVE
           (is_equal against a column-iota constant), and accumulate
           out_tile = sum_g onehot_g.T @ Xrows_g in PSUM. One DMA per tile
           writes the finished [128, 1024] block of the output.

All shapes/counts are identical across cores (SPMD); per-core data
differences live entirely in the input tensors (Yt, gather indices,
relative-column tables).
"""

import os

import numpy as np

import concourse.bacc as bacc
import concourse.mybir as mybir
import concourse.tile as tile
from concourse.bass_utils import run_bass_kernel_spmd

# Problem constants (hardcoded per harness contract).
B = 8
HEADS = 16
K = 1024
HEAD_DIM = 128
OUT_DIM = 1024
T_SLOTS = 4096

R = HEADS * K            # rows per batch = 16384
NT = T_SLOTS // 128      # output tiles per batch = 32
PT = 640                 # gather buffer rows per output tile (5 groups)
NG = PT // 128           # row groups (matmuls) per output tile = 5
NCORES = 8

F32 = mybir.dt.float32
F32R = mybir.dt.float32r
BF16 = mybir.dt.bfloat16
FP16 = mybir.dt.float16
I16 = mybir.dt.int16

# Projection matmul dtype: "f32r" (full-rate fp32), "f32" (4x slower),
# or "fp16" (halves Y/W traffic, ~2x err).
MM_DTYPE = os.environ.get("ES_MM_DTYPE", "fp16")
MM_F32R = MM_DTYPE == "f32r"
# X staging / scatter dtype: "fp16", "bf16", "f32r", or "f32".
X_DTYPE = os.environ.get("ES_X_DTYPE", "fp16")
# Debug: which phases to emit ("AB", "A", "B").
PHASES = os.environ.get("ES_PHASES", "AB")
# Scheduling knobs.
GBUFS = int(os.environ.get("ES_GBUFS", "4"))
XBUFS = int(os.environ.get("ES_XBUFS", "6"))
WSPLIT = os.environ.get("ES_WSPLIT", "1") == "1"
BARRIER = os.environ.get("ES_BARRIER", "0") == "1"
# Write the output in fp16 (host casts back to f32): halves out traffic.
OUT_FP16 = os.environ.get("ES_OUT_FP16", "1") == "1"
# Alternate PSUM->SBUF copies between DVE and ACT.
ALT_COPY = os.environ.get("ES_ALT_COPY", "1") == "1"
# Trailing -1 index padding (skipped by Q7 -> less gather traffic). Found
# unreliable on HW at full scale (intermittent NRT faults) -> default off.
EXACT_CNT = os.environ.get("ES_EXACT_CNT", "0") == "1"

_cache = {}


def _build_program(mdt, sdt, gnum):
    """mdt: projection matmul dtype; sdt: X staging + scatter dtype;
    gnum: gathered positions per tile (<= PT; rest is never read thanks to
    the one-hot sentinel, but must hold finite values)."""
    nc = bacc.Bacc("TRN2", target_bir_lowering=False, debug=False,
                   num_devices=NCORES)

    yt = nc.dram_tensor("yt", [HEAD_DIM, R], mdt, kind="ExternalInput").ap()
    w = nc.dram_tensor("w", [HEAD_DIM, HEADS * OUT_DIM], mdt,
                       kind="ExternalInput").ap()
    gidx = nc.dram_tensor("gidx", [128, NT * (PT // 16)], I16,
                          kind="ExternalInput").ap()
    relc = nc.dram_tensor("relc", [128, NT * NG], F32,
                          kind="ExternalInput").ap()
    cols = nc.dram_tensor("cols", [128, 128], F32, kind="ExternalInput").ap()
    odt = FP16 if OUT_FP16 else F32
    out = nc.dram_tensor("out", [T_SLOTS, OUT_DIM], odt,
                         kind="ExternalOutput").ap()
    xnat = nc.dram_tensor("xnat", [R, OUT_DIM], sdt).ap()

    with tile.TileContext(nc) as tc:
        with (
            tc.tile_pool(name="const", bufs=1) as cpool,
            tc.tile_pool(name="yhead",
                         bufs=int(os.environ.get("ES_YBUFS", "2"))) as ypool,
            tc.tile_pool(name="xchunk", bufs=XBUFS) as xpool,
            tc.tile_pool(name="gather", bufs=GBUFS) as gpool,
            tc.tile_pool(name="onehot",
                         bufs=int(os.environ.get("ES_OHBUFS", "4"))) as ohpool,
            tc.tile_pool(name="otile",
                         bufs=int(os.environ.get("ES_OBUFS", "4"))) as opool,
        ):
            w_sb = cpool.tile([128, HEADS * OUT_DIM], mdt, tag="w")
            if WSPLIT:
                for e in range(HEADS):
                    nc.sync.dma_start(
                        out=w_sb[:, e * OUT_DIM:(e + 1) * OUT_DIM],
                        in_=w[:, e * OUT_DIM:(e + 1) * OUT_DIM])
            else:
                nc.sync.dma_start(out=w_sb[:], in_=w[:])
            gidx_sb = cpool.tile([128, NT * (PT // 16)], I16, tag="gidx")
            nc.sync.dma_start(out=gidx_sb[:], in_=gidx[:])
            relc_sb = cpool.tile([128, NT * NG], F32, tag="relc")
            nc.sync.dma_start(out=relc_sb[:], in_=relc[:])
            cols_sb = cpool.tile([128, 128], F32, tag="cols")
            nc.sync.dma_start(out=cols_sb[:], in_=cols[:])

            # ---- Phase A: projection, X written to HBM in natural order --
            pa_ctx = tc.tile_pool(name="psumA",
                                  bufs=int(os.environ.get("ES_PABUFS", "2")),
                                  space="PSUM")
            pspool = pa_ctx.__enter__()
            for e in range(HEADS if "A" in PHASES else 0):
                yt_e = ypool.tile([128, K], mdt, tag="yt")
                nc.sync.dma_start(out=yt_e[:], in_=yt[:, e * K:(e + 1) * K])
                for rc in range(K // 128):
                    px = pspool.tile([128, OUT_DIM], F32, tag="pa")
                    lhsT = yt_e[:, rc * 128:(rc + 1) * 128]
                    for h in range(2):
                        nc.tensor.matmul(
                            out=px[:, h * 512:(h + 1) * 512],
                            lhsT=lhsT,
                            rhs=w_sb[:, e * OUT_DIM + h * 512:
                                     e * OUT_DIM + (h + 1) * 512],
                            start=True, stop=True,
                        )
                    xc = xpool.tile([128, OUT_DIM], sdt, tag="xc")
                    if ALT_COPY and rc % 2 == 1:
                        nc.scalar.copy(out=xc[:], in_=px[:])
                    else:
                        nc.vector.tensor_copy(out=xc[:], in_=px[:])
                    row0 = (e * (K // 128) + rc) * 128
                    xeng = (nc.scalar if os.environ.get("ES_DMAALT", "0") == "1"
                            and rc % 2 == 0 else nc.sync)
                    xeng.dma_start(out=xnat[row0:row0 + 128, :], in_=xc[:])

            pa_ctx.__exit__(None, None, None)

            # Fence: every gather below reads rows written above.
            if BARRIER and "A" in PHASES and "B" in PHASES:
                tc.strict_bb_all_engine_barrier()
            pb_ctx = tc.tile_pool(name="psumB",
                                  bufs=int(os.environ.get("ES_PBBUFS", "2")),
                                  space="PSUM")
            pspool = pb_ctx.__enter__()

            # ---- Phase B: gather sorted rows per tile, one-hot matmul ----
            splitg = os.environ.get("ES_SPLITG", "1") == "1"
            for t in range(NT if "B" in PHASES else 0):
                g = gpool.tile([128, NG, OUT_DIM], sdt, tag="g")
                if EXACT_CNT and t < GBUFS:
                    # With -1 skip-padding, unwritten positions vary per
                    # tile; scrub whole fresh slots once so unread regions
                    # hold finite values (one-hot sentinel zeroes them).
                    nc.gpsimd.memset(g[:], 0.0)
                elif gnum < PT and t < GBUFS:
                    # Positions gnum..PT are never gathered; scrub the
                    # fresh SBUF slots once so the unread region holds
                    # finite values (one-hot sentinel zeroes them out).
                    lastp = (gnum // 128) * 128
                    nc.gpsimd.memset(g[gnum - lastp:, NG - 1, :], 0.0)
                gq = (t % 2) if os.environ.get("ES_GQALT", "0") == "1" else 0
                if splitg:
                    cut = int(os.environ.get("ES_GCUT", "384"))
                    nc.gpsimd.dma_gather(
                        out_ap=g[:, 0:cut // 128, :],
                        in_ap=xnat[:],
                        idxs_ap=gidx_sb[:, t * (PT // 16):
                                        t * (PT // 16) + cut // 16],
                        num_idxs=cut, num_idxs_reg=cut, elem_size=OUT_DIM,
                        queue_num=gq,
                    )
                    nc.gpsimd.dma_gather(
                        out_ap=g[:, cut // 128:NG, :],
                        in_ap=xnat[:],
                        idxs_ap=gidx_sb[:, t * (PT // 16) + cut // 16:
                                        t * (PT // 16) + gnum // 16],
                        num_idxs=gnum - cut, num_idxs_reg=gnum - cut,
                        elem_size=OUT_DIM, queue_num=gq,
                    )
                else:
                    nc.gpsimd.dma_gather(
                        out_ap=g[:],
                        in_ap=xnat[:],
                        idxs_ap=gidx_sb[:, t * (PT // 16):
                                        t * (PT // 16) + gnum // 16],
                        num_idxs=gnum,
                        num_idxs_reg=gnum,
                        elem_size=OUT_DIM,
                    )
                pt = pspool.tile([128, OUT_DIM], F32, tag="pb")
                for gi in range(NG):
                    oh = ohpool.tile([128, 128], sdt, tag="oh")
                    c = t * NG + gi
                    nc.vector.tensor_tensor(
                        out=oh[:],
                        in0=relc_sb[:, c:c + 1].to_broadcast([128, 128]),
                        in1=cols_sb[:],
                        op=mybir.AluOpType.is_equal,
                    )
                    for h in range(2):
                        nc.tensor.matmul(
                            out=pt[:, h * 512:(h + 1) * 512],
                            lhsT=oh[:],
                            rhs=g[:, gi, h * 512:(h + 1) * 512],
                            start=(gi == 0), stop=(gi == NG - 1),
                        )
                ot = opool.tile([128, OUT_DIM], odt, tag="ot")
                if ALT_COPY and t % 2 == 1:
                    nc.scalar.copy(out=ot[:], in_=pt[:])
                else:
                    nc.vector.tensor_copy(out=ot[:], in_=pt[:])
                nc.sync.dma_start(out=out[t * 128:(t + 1) * 128, :], in_=ot[:])
            pb_ctx.__exit__(None, None, None)

    nc.compile()
    return nc


def _get_program(gnum=576):
    mdt = {"f32r": F32R, "f32": F32, "fp16": FP16, "bf16": BF16}[MM_DTYPE]
    sdt = {"f32r": F32R if MM_F32R else F32, "f32": F32,
           "bf16": BF16, "fp16": FP16}[X_DTYPE]
    key = (MM_DTYPE, X_DTYPE, PHASES, GBUFS, XBUFS, WSPLIT, BARRIER,
           ALT_COPY, EXACT_CNT, OUT_FP16, gnum,
           os.environ.get("ES_SPLITG", "1"),
           os.environ.get("ES_OBUFS", "4"), os.environ.get("ES_YBUFS", "2"),
           os.environ.get("ES_PABUFS", "2"), os.environ.get("ES_PBBUFS", "2"))
    if key not in _cache:
        _cache[key] = _build_program(mdt, sdt, gnum)
    return _cache[key]


def _prep_core_inputs(Yb, Indb):
    """Host-side prep for one batch: transpose Y, sort rows by slot,
    build padded gather-index and relative-column tables."""
    yt = np.ascontiguousarray(
        Yb.transpose(2, 0, 1).reshape(HEAD_DIM, R)).astype(np.float32)
    ind = Indb.reshape(R).astype(np.int64)
    order = np.argsort(ind, kind="stable")
    sind = ind[order]
    counts = np.bincount(sind // 128, minlength=NT)
    assert counts.max() <= PT, f"tile overflow: {counts.max()} > {PT}"
    _prep_core_inputs.max_count = max(
        getattr(_prep_core_inputs, "max_count", 0), int(counts.max()))
    pad = -1 if EXACT_CNT else 0
    gidx = np.full((NT, PT), pad, dtype=np.int16)
    relc = np.full((NT, PT), -1000.0, dtype=np.float32)
    pos = 0
    for t in range(NT):
        c = counts[t]
        gidx[t, :c] = order[pos:pos + c]
        relc[t, :c] = (sind[pos:pos + c] - t * 128).astype(np.float32)
        pos += c
    # dma_gather index layout: position p -> (partition p%16, col p//16),
    # and the 16-partition block replicated across all 8 Q7 core groups.
    blk = np.concatenate(
        [gidx[t].reshape(PT // 16, 16).T for t in range(NT)], axis=1)
    gidx_sb = np.ascontiguousarray(np.tile(blk, (8, 1)), dtype=np.int16)
    # one-hot layout: position p -> (partition p%128, group p//128)
    relc_sb = np.concatenate(
        [relc[t].reshape(NG, 128).T for t in range(NT)], axis=1)
    relc_sb = np.ascontiguousarray(relc_sb, dtype=np.float32)
    return yt, gidx_sb, relc_sb


def kernel(Y, Ind, T, W):
    Y = np.asarray(Y, dtype=np.float32)
    Ind = np.asarray(Ind)
    W = np.asarray(W, dtype=np.float32)
    assert int(T) == T_SLOTS and Y.shape == (B, HEADS, K, HEAD_DIM)

    if MM_DTYPE == "fp16":
        np_mdt = np.float16
    elif MM_DTYPE == "bf16":
        import ml_dtypes
        np_mdt = ml_dtypes.bfloat16
    else:
        np_mdt = np.float32
    w_in = np.ascontiguousarray(
        W.transpose(1, 0, 2).reshape(HEAD_DIM, HEADS * OUT_DIM)
    ).astype(np_mdt)
    cols_in = np.broadcast_to(
        np.arange(128, dtype=np.float32)[None, :], (128, 128)).copy()

    _prep_core_inputs.max_count = 0
    in_maps = []
    for b in range(B):
        yt, gidx_sb, relc_sb = _prep_core_inputs(Y[b], Ind[b])
        in_maps.append({
            "yt": yt.astype(np_mdt), "w": w_in, "gidx": gidx_sb,
            "relc": relc_sb, "cols": cols_in,
        })
    gnum = 576 if _prep_core_inputs.max_count <= 576 else PT
    nc = _get_program(gnum)

    # The first execution of a freshly compiled NEFF occasionally wedges a
    # core (NRT_EXEC_UNIT_UNRECOVERABLE); a retry on a fresh execute has
    # been observed to recover.
    last_exc = None
    for attempt in range(3):
        try:
            res = run_bass_kernel_spmd(
                nc, in_maps, core_ids=list(range(NCORES)),
                trace=os.environ.get("ES_TRACE", "0") == "1",
            )
            break
        except Exception as exc:  # noqa: BLE001 - device flake, retry
            last_exc = exc
            import time as _time
            _time.sleep(2.0)
    else:
        raise last_exc
    kernel.last_results = res
    out = np.stack([res.results[b]["out"] for b in range(B)], axis=0)
    return out.astype(np.float32)



# revision 4
# speedup vs baseline: 1.5486x; 1.5486x over previous
"""ExpertScatter TRN2 kernel — direct DMA scatter-add.

reference semantics:
    X = einsum('bekj,eji->beki', Y, W)          # per-head projection
    out[b] = zeros([T, I]); out[b, Ind[b,e,k]] += X[b,e,k]

Strategy (data-parallel over batch, 1 batch per NeuronCore):
  The output accumulator lives in HBM (fp16). Per head e, project
  Yt chunks against W[e] on the PE (fp16 operands, fp32 PSUM), copy
  each [128, 1024] X chunk to SBUF (fp16), and scatter-add the head's
  1024 rows straight into the output with gpsimd.dma_scatter_add
  (out[idx_i] += x_row_i).

  Duplicate-index hazard: descriptors of ONE scatter call race on HBM
  read-modify-write (probed: duplicates closer than ~256 positions lose
  adds). Fix: the host pre-sums rows of the same (head, slot) — exact,
  since they share W[e] — so each per-head call has unique indices
  (~906 real + zero-payload pad rows, idx 0). Cross-head duplicates are
  safe: the 16 calls are WAW-serialized on the out tensor by the tile
  framework (next call's transfer waits the previous DMA-completion
  semaphore).

  Total per-core DMA: Y 4MB + W 4MB + zero-init 8MB + scatter 32MB
  = 48MB, vs ~80MB for a sort+stage+gather pipeline. No phase B: no
  global sort, no gather, no one-hot matmuls.
"""

import os

import numpy as np

import concourse.bacc as bacc
import concourse.mybir as mybir
import concourse.tile as tile
from concourse.bass_utils import run_bass_kernel_spmd

# Problem constants (hardcoded per harness contract).
B = 8
HEADS = 16
K = 1024
HEAD_DIM = 128
OUT_DIM = 1024
T_SLOTS = 4096
NCORES = 8

NG = K // 128                             # X groups per head = 8

F32 = mybir.dt.float32
F16 = mybir.dt.float16
I16 = mybir.dt.int16

XBUFS = int(os.environ.get("ES_XBUFS", "3"))
PBUFS = int(os.environ.get("ES_PBUFS", "3"))
YBUFS = int(os.environ.get("ES_YBUFS", "4"))

_cache = {}


def _build_program():
    nc = bacc.Bacc("TRN2", target_bir_lowering=False, debug=False,
                   num_devices=NCORES)

    yt = nc.dram_tensor("yt", [HEAD_DIM, HEADS * K], F16,
                        kind="ExternalInput").ap()
    w = nc.dram_tensor("w", [HEAD_DIM, HEADS * OUT_DIM], F16,
                       kind="ExternalInput").ap()
    idx = nc.dram_tensor("idx", [128, HEADS * K // 16], I16,
                         kind="ExternalInput").ap()
    out = nc.dram_tensor("out", [T_SLOTS, OUT_DIM], F16,
                         kind="ExternalOutput").ap()

    with tile.TileContext(nc) as tc:
        with (
            tc.tile_pool(name="const", bufs=1) as cpool,
            tc.tile_pool(name="yhead", bufs=YBUFS) as ypool,
            tc.tile_pool(name="xblk", bufs=XBUFS) as xpool,
            tc.tile_pool(name="psum", bufs=PBUFS, space="PSUM") as pspool,
        ):
            # Constants / zero tile.
            z = cpool.tile([128, OUT_DIM], F16, tag="z")
            nc.vector.memset(z[:], 0.0)
            idx_sb = cpool.tile([128, HEADS * K // 16], I16, tag="idx")
            nc.sync.dma_start(out=idx_sb[:], in_=idx[:])
            w_sb = cpool.tile([128, HEADS * OUT_DIM], F16, tag="w")
            for e in range(HEADS):
                nc.sync.dma_start(
                    out=w_sb[:, e * OUT_DIM:(e + 1) * OUT_DIM],
                    in_=w[:, e * OUT_DIM:(e + 1) * OUT_DIM])

            # Zero-init the HBM accumulator (cheap Pool-queue issues).
            for t in range(T_SLOTS // 128):
                nc.gpsimd.dma_start(out=out[t * 128:(t + 1) * 128, :],
                                    in_=z[:])

            for e in range(HEADS):
                xb = xpool.tile([128, NG, OUT_DIM], F16, tag="xb")
                yt_e = ypool.tile([128, K], F16, tag="yt")
                nc.sync.dma_start(out=yt_e[:],
                                  in_=yt[:, e * K:(e + 1) * K])
                for rc in range(NG):
                    px = pspool.tile([128, OUT_DIM], F32, tag="px")
                    for h in range(2):
                        nc.tensor.matmul(
                            out=px[:, h * 512:(h + 1) * 512],
                            lhsT=yt_e[:, rc * 128:(rc + 1) * 128],
                            rhs=w_sb[:, e * OUT_DIM + h * 512:
                                     e * OUT_DIM + (h + 1) * 512],
                            start=True, stop=True,
                        )
                    if rc % 2 == 0:
                        nc.vector.tensor_copy(out=xb[:, rc, :], in_=px[:])
                    else:
                        nc.scalar.copy(out=xb[:, rc, :], in_=px[:])
                nc.gpsimd.dma_scatter_add(
                    out[:],
                    xb[:],
                    idx_sb[:, e * (K // 16):(e + 1) * (K // 16)],
                    K,
                    K,
                    OUT_DIM,
                )

    nc.compile()
    return nc


def _get_program():
    if "nc" not in _cache:
        _cache["nc"] = _build_program()
    return _cache["nc"]


def _dedup_head(Yh, indh):
    """Pre-sum rows sharing a slot (exact: same W). Returns (Y', idx')
    with unique slots first, zero rows + idx 0 padding after."""
    order = np.argsort(indh, kind="stable")
    sind = indh[order]
    starts = np.concatenate(([0], np.nonzero(np.diff(sind))[0] + 1))
    sums = np.add.reduceat(Yh[order].astype(np.float32), starts, axis=0)
    cnt = len(starts)
    Yd = np.zeros((K, HEAD_DIM), dtype=np.float32)
    idxd = np.zeros(K, dtype=np.int16)
    Yd[:cnt] = sums
    idxd[:cnt] = sind[starts]
    return Yd, idxd


def kernel(Y, Ind, T, W):
    Y = np.asarray(Y, dtype=np.float32)
    Ind = np.asarray(Ind)
    W = np.asarray(W, dtype=np.float32)
    assert int(T) == T_SLOTS and Y.shape == (B, HEADS, K, HEAD_DIM)

    w_in = np.ascontiguousarray(
        W.transpose(1, 0, 2).reshape(HEAD_DIM, HEADS * OUT_DIM)
    ).astype(np.float16)

    in_maps = []
    for b in range(B):
        ytb = np.empty((HEAD_DIM, HEADS * K), dtype=np.float16)
        idx_cols = []
        for e in range(HEADS):
            Yd, idxd = _dedup_head(Y[b, e], Ind[b, e].astype(np.int64))
            ytb[:, e * K:(e + 1) * K] = Yd.T.astype(np.float16)
            idx_cols.append(np.tile(idxd.reshape(K // 16, 16).T, (8, 1)))
        idx_in = np.ascontiguousarray(np.concatenate(idx_cols, axis=1),
                                      dtype=np.int16)
        in_maps.append({"yt": ytb, "w": w_in, "idx": idx_in})

    nc = _get_program()

    # The first execution of a freshly compiled NEFF occasionally wedges a
    # core (NRT_EXEC_UNIT_UNRECOVERABLE); a retry on a fresh execute has
    # been observed to recover.
    last_exc = None
    for attempt in range(3):
        try:
            res = run_bass_kernel_spmd(
                nc, in_maps, core_ids=list(range(NCORES)),
                trace=os.environ.get("ES_TRACE", "0") == "1",
            )
            break
        except Exception as exc:  # noqa: BLE001 - device flake, retry
            last_exc = exc
            import time as _time
            _time.sleep(2.0)
    else:
        raise last_exc
    kernel.last_results = res
    out = np.stack([res.results[b]["out"] for b in range(B)], axis=0)
    return out.astype(np.float32)


# revision 10
# speedup vs baseline: 1.8870x; 1.2185x over previous
"""ExpertScatter TRN2 kernel — direct DMA scatter-add, parity-split outputs.

reference semantics:
    X = einsum('bekj,eji->beki', Y, W)          # per-head projection
    out[b] = zeros([T, I]); out[b, Ind[b,e,k]] += X[b,e,k]

Strategy (data-parallel over batch, 1 batch per NeuronCore):
  Per head e, project Yt chunks against W[e] on the PE (fp16 operands,
  fp32 PSUM), copy each [128, 1024] X chunk to SBUF (fp16), and
  scatter-add the rows straight into HBM output accumulators with
  gpsimd.dma_scatter_add (out[idx_i] += x_row_i).

  Duplicate-index hazard: descriptors of ONE scatter call race on HBM
  read-modify-write (probed: duplicates closer than ~256 positions lose
  adds). Fix: the host pre-sums rows of the same (head, slot) — exact,
  since they share W[e] — so each call has unique indices. Cross-head
  duplicates are safe: calls targeting the same tensor are
  WAW-serialized by the tile framework (next call's transfer waits the
  previous DMA-completion semaphore).

  The WAW chain leaves a ~3us bubble between consecutive scatter calls
  (sem propagation + Q7 descriptor-gen + DGE trigger delay). To fill
  those bubbles, the output is split by slot parity into TWO separate
  DRAM tensors (out_even = slots 0,2,..., out_odd = slots 1,3,...) with
  independent WAW chains; their transfers interleave on the shared DMA
  engines, and the host re-interleaves rows at the end (free).

  Total per-core DMA: Y 4MB + W 4MB + zero-init 8MB + scatter 32MB
  = 48MB, vs ~80MB for a sort+stage+gather pipeline. No global sort, no
  gather, no one-hot matmuls.
"""

import os

import numpy as np

import concourse.bacc as bacc
import concourse.mybir as mybir
import concourse.tile as tile
from concourse.bass_utils import run_bass_kernel_spmd

# Problem constants (hardcoded per harness contract).
B = 8
HEADS = 16
K = 1024
HEAD_DIM = 128
OUT_DIM = 1024
T_SLOTS = 4096
NCORES = 8

PP = 512                 # padded rows per (head, parity); 487 on seed-0 data

F32 = mybir.dt.float32
F16 = mybir.dt.float16
I16 = mybir.dt.int16

XBUFS = int(os.environ.get("ES_XBUFS", "3"))
PBUFS = int(os.environ.get("ES_PBUFS", "3"))
YBUFS = int(os.environ.get("ES_YBUFS", "4"))

_cache = {}


def _build_program(pp, caps):
    ng = (2 * pp) // 128                  # X groups per head
    gpar = ng // 2                        # groups per parity
    kh = 2 * pp                           # padded rows per head
    nc = bacc.Bacc("TRN2", target_bir_lowering=False, debug=False,
                   num_devices=NCORES)

    yt = nc.dram_tensor("yt", [HEAD_DIM, HEADS * kh], F16,
                        kind="ExternalInput").ap()
    w = nc.dram_tensor("w", [HEAD_DIM, HEADS * OUT_DIM], F16,
                       kind="ExternalInput").ap()
    idx = nc.dram_tensor("idx", [128, HEADS * kh // 16], I16,
                         kind="ExternalInput").ap()
    outs = [nc.dram_tensor(nm, [T_SLOTS // 2, OUT_DIM], F16,
                           kind="ExternalOutput").ap()
            for nm in ("out_even", "out_odd")]

    with tile.TileContext(nc) as tc:
        with (
            tc.tile_pool(name="const", bufs=1) as cpool,
            tc.tile_pool(name="yhead", bufs=YBUFS) as ypool,
            tc.tile_pool(name="xblk", bufs=XBUFS) as xpool,
            tc.tile_pool(name="psum", bufs=PBUFS, space="PSUM") as pspool,
        ):
            z = cpool.tile([128, OUT_DIM], F16, tag="z")
            nc.vector.memset(z[:], 0.0)
            idx_sb = cpool.tile([128, HEADS * kh // 16], I16, tag="idx")
            nc.sync.dma_start(out=idx_sb[:], in_=idx[:])
            w_sb = cpool.tile([128, HEADS * OUT_DIM], F16, tag="w")

            def load_w(e):
                nc.sync.dma_start(
                    out=w_sb[:, e * OUT_DIM:(e + 1) * OUT_DIM],
                    in_=w[:, e * OUT_DIM:(e + 1) * OUT_DIM])

            yts = {}

            def load_yt(e):
                yt_e = ypool.tile([128, kh], F16, tag="yt")
                nc.sync.dma_start(out=yt_e[:],
                                  in_=yt[:, e * kh:(e + 1) * kh])
                yts[e] = yt_e

            # Early loads so head-0/1 compute overlaps the zero-init.
            for e in range(2):
                load_w(e)
                load_yt(e)
            # Zero-init the accumulators: even chain issued from SP (early),
            # odd chain from ACT; both ahead of their first scatters.
            for t in range(T_SLOTS // 256):
                nc.sync.dma_start(out=outs[0][t * 128:(t + 1) * 128, :],
                                  in_=z[:])
            for t in range(T_SLOTS // 256):
                nc.scalar.dma_start(out=outs[1][t * 128:(t + 1) * 128, :],
                                    in_=z[:])
            for e in range(2, HEADS):
                load_w(e)

            for e in range(HEADS):
                xb = xpool.tile([128, ng, OUT_DIM], F16, tag="xb")
                if e not in yts:
                    load_yt(e)
                yt_e = yts[e]
                for rc in range(ng):
                    px = pspool.tile([128, OUT_DIM], F32, tag="px")
                    for h in range(2):
                        nc.tensor.matmul(
                            out=px[:, h * 512:(h + 1) * 512],
                            lhsT=yt_e[:, rc * 128:(rc + 1) * 128],
                            rhs=w_sb[:, e * OUT_DIM + h * 512:
                                     e * OUT_DIM + (h + 1) * 512],
                            start=True, stop=True,
                        )
                    if rc % 2 == 0:
                        nc.vector.tensor_copy(out=xb[:, rc, :], in_=px[:])
                    else:
                        nc.scalar.copy(out=xb[:, rc, :], in_=px[:])
                for par in range(2):
                    cap = caps[e * 2 + par]
                    nc.gpsimd.dma_scatter_add(
                        outs[par][:],
                        xb[:, par * gpar:(par + 1) * gpar, :],
                        idx_sb[:, (e * 2 + par) * (pp // 16):
                               (e * 2 + par) * (pp // 16) + cap // 16],
                        cap,
                        cap,
                        OUT_DIM,
                    )

    nc.compile()
    return nc


def _get_program(pp, caps):
    key = (pp, caps)
    if key not in _cache:
        _cache[key] = _build_program(pp, caps)
    return _cache[key]


def _dedup_head(Yh, indh):
    """Pre-sum rows sharing a slot (exact: same W)."""
    order = np.argsort(indh, kind="stable")
    sind = indh[order]
    starts = np.concatenate(([0], np.nonzero(np.diff(sind))[0] + 1))
    sums = np.add.reduceat(Yh[order].astype(np.float32), starts, axis=0)
    return sums, sind[starts]


def kernel(Y, Ind, T, W):
    Y = np.asarray(Y, dtype=np.float32)
    Ind = np.asarray(Ind)
    W = np.asarray(W, dtype=np.float32)
    assert int(T) == T_SLOTS and Y.shape == (B, HEADS, K, HEAD_DIM)

    w_in = np.ascontiguousarray(
        W.transpose(1, 0, 2).reshape(HEAD_DIM, HEADS * OUT_DIM)
    ).astype(np.float16)

    # Host prep: dedup per head, split by slot parity.
    per_core = []
    cnts = np.zeros((B, HEADS * 2), dtype=np.int64)
    for b in range(B):
        heads = []
        for e in range(HEADS):
            sums, slots = _dedup_head(Y[b, e], Ind[b, e].astype(np.int64))
            parts = []
            for par in range(2):
                m = (slots % 2) == par
                parts.append((sums[m], (slots[m] // 2).astype(np.int16)))
                cnts[b, e * 2 + par] = int(m.sum())
            heads.append(parts)
        per_core.append(heads)
    maxcnt = int(cnts.max())
    pp = max(PP, -(-maxcnt // 128) * 128)
    kh = 2 * pp
    # Per-call num_idxs cap: shared across cores (SPMD), tight per call.
    caps = tuple(int(-(-int(c) // 16) * 16) or 16
                 for c in cnts.max(axis=0))

    in_maps = []
    for b in range(B):
        ytb = np.zeros((HEAD_DIM, HEADS * kh), dtype=np.float16)
        idx_cols = []
        for e in range(HEADS):
            for par in range(2):
                rows, slots = per_core[b][e][par]
                cnt = len(slots)
                base = e * kh + par * pp
                ytb[:, base:base + cnt] = rows.T.astype(np.float16)
                idxp = np.zeros(pp, dtype=np.int16)
                idxp[:cnt] = slots
                idx_cols.append(np.tile(idxp.reshape(pp // 16, 16).T, (8, 1)))
        idx_in = np.ascontiguousarray(np.concatenate(idx_cols, axis=1),
                                      dtype=np.int16)
        in_maps.append({"yt": ytb, "w": w_in, "idx": idx_in})

    nc = _get_program(pp)

    # The first execution of a freshly compiled NEFF occasionally wedges a
    # core (NRT_EXEC_UNIT_UNRECOVERABLE); a retry on a fresh execute has
    # been observed to recover.
    last_exc = None
    for attempt in range(3):
        try:
            res = run_bass_kernel_spmd(
                nc, in_maps, core_ids=list(range(NCORES)),
                trace=os.environ.get("ES_TRACE", "0") == "1",
            )
            break
        except Exception as exc:  # noqa: BLE001 - device flake, retry
            last_exc = exc
            import time as _time
            _time.sleep(2.0)
    else:
        raise last_exc
    kernel.last_results = res
    out = np.empty((B, T_SLOTS, OUT_DIM), dtype=np.float32)
    for b in range(B):
        out[b, 0::2] = res.results[b]["out_even"].astype(np.float32)
        out[b, 1::2] = res.results[b]["out_odd"].astype(np.float32)
    return out


# revision 12
# speedup vs baseline: 2.0328x; 1.0773x over previous
"""ExpertScatter TRN2 kernel — direct DMA scatter-add, parity-split outputs.

reference semantics:
    X = einsum('bekj,eji->beki', Y, W)          # per-head projection
    out[b] = zeros([T, I]); out[b, Ind[b,e,k]] += X[b,e,k]

Strategy (data-parallel over batch, 1 batch per NeuronCore):
  Per head e, project Yt chunks against W[e] on the PE (fp16 operands,
  fp32 PSUM), copy each [128, 1024] X chunk to SBUF (fp16), and
  scatter-add the rows straight into HBM output accumulators with
  gpsimd.dma_scatter_add (out[idx_i] += x_row_i).

  Duplicate-index hazard: descriptors of ONE scatter call race on HBM
  read-modify-write (probed: duplicates closer than ~256 positions lose
  adds). Fix: the host pre-sums rows of the same (head, slot) — exact,
  since they share W[e] — so each call has unique indices. Cross-head
  duplicates are safe: calls targeting the same tensor are
  WAW-serialized by the tile framework (next call's transfer waits the
  previous DMA-completion semaphore).

  The WAW chain leaves a ~3us bubble between consecutive scatter calls
  (sem propagation + Q7 descriptor-gen + DGE trigger delay). To fill
  those bubbles, the output is split by slot parity into TWO separate
  DRAM tensors (out_even = slots 0,2,..., out_odd = slots 1,3,...) with
  independent WAW chains; their transfers interleave on the shared DMA
  engines, and the host re-interleaves rows at the end (free).

  Total per-core DMA: Y 4MB + W 4MB + zero-init 8MB + scatter 32MB
  = 48MB, vs ~80MB for a sort+stage+gather pipeline. No global sort, no
  gather, no one-hot matmuls.
"""

import os

import numpy as np

import concourse.bacc as bacc
import concourse.mybir as mybir
import concourse.tile as tile
from concourse.bass_utils import run_bass_kernel_spmd

# Problem constants (hardcoded per harness contract).
B = 8
HEADS = 16
K = 1024
HEAD_DIM = 128
OUT_DIM = 1024
T_SLOTS = 4096
NCORES = 8

PP = 512                 # padded rows per (head, parity); 487 on seed-0 data

F32 = mybir.dt.float32
F16 = mybir.dt.float16
I16 = mybir.dt.int16

XBUFS = int(os.environ.get("ES_XBUFS", "3"))
PBUFS = int(os.environ.get("ES_PBUFS", "3"))
YBUFS = int(os.environ.get("ES_YBUFS", "4"))

_cache = {}


def _build_program(pp, caps):
    ng = (2 * pp) // 128                  # X groups per head
    gpar = ng // 2                        # groups per parity
    kh = 2 * pp                           # padded rows per head
    nc = bacc.Bacc("TRN2", target_bir_lowering=False, debug=False,
                   num_devices=NCORES, num_swdge_queues=2,
                   dynamic_dma_scratch_size=131072)

    yt = nc.dram_tensor("yt", [HEAD_DIM, HEADS * kh], F16,
                        kind="ExternalInput").ap()
    w = nc.dram_tensor("w", [HEAD_DIM, HEADS * OUT_DIM], F16,
                       kind="ExternalInput").ap()
    idx = nc.dram_tensor("idx", [128, HEADS * kh // 16], I16,
                         kind="ExternalInput").ap()
    outs = [nc.dram_tensor(nm, [T_SLOTS // 2, OUT_DIM], F16,
                           kind="ExternalOutput").ap()
            for nm in ("out_even", "out_odd")]

    with tile.TileContext(nc) as tc:
        with (
            tc.tile_pool(name="const", bufs=1) as cpool,
            tc.tile_pool(name="yhead", bufs=YBUFS) as ypool,
            tc.tile_pool(name="xblk", bufs=XBUFS) as xpool,
            tc.tile_pool(name="psum", bufs=PBUFS, space="PSUM") as pspool,
        ):
            z = cpool.tile([128, OUT_DIM], F16, tag="z")
            nc.vector.memset(z[:], 0.0)
            idx_sb = cpool.tile([128, HEADS * kh // 16], I16, tag="idx")
            nc.sync.dma_start(out=idx_sb[:], in_=idx[:])
            w_sb = cpool.tile([128, HEADS * OUT_DIM], F16, tag="w")

            def load_w(e):
                nc.sync.dma_start(
                    out=w_sb[:, e * OUT_DIM:(e + 1) * OUT_DIM],
                    in_=w[:, e * OUT_DIM:(e + 1) * OUT_DIM])

            yts = {}

            def load_yt(e):
                yt_e = ypool.tile([128, kh], F16, tag="yt")
                nc.sync.dma_start(out=yt_e[:],
                                  in_=yt[:, e * kh:(e + 1) * kh])
                yts[e] = yt_e

            # Early loads so head-0/1 compute overlaps the zero-init.
            for e in range(2):
                load_w(e)
                load_yt(e)
            # Zero-init the accumulators: even chain issued from SP (early),
            # odd chain from ACT; both ahead of their first scatters.
            for t in range(T_SLOTS // 256):
                nc.sync.dma_start(out=outs[0][t * 128:(t + 1) * 128, :],
                                  in_=z[:])
            for t in range(T_SLOTS // 256):
                nc.scalar.dma_start(out=outs[1][t * 128:(t + 1) * 128, :],
                                    in_=z[:])
            for e in range(2, HEADS):
                load_w(e)

            for e in range(HEADS):
                xb = xpool.tile([128, ng, OUT_DIM], F16, tag="xb")
                if e not in yts:
                    load_yt(e)
                yt_e = yts[e]
                for rc in range(ng):
                    px = pspool.tile([128, OUT_DIM], F32, tag="px")
                    for h in range(2):
                        nc.tensor.matmul(
                            out=px[:, h * 512:(h + 1) * 512],
                            lhsT=yt_e[:, rc * 128:(rc + 1) * 128],
                            rhs=w_sb[:, e * OUT_DIM + h * 512:
                                     e * OUT_DIM + (h + 1) * 512],
                            start=True, stop=True,
                        )
                    if rc % 2 == 0:
                        nc.vector.tensor_copy(out=xb[:, rc, :], in_=px[:])
                    else:
                        nc.scalar.copy(out=xb[:, rc, :], in_=px[:])
                for par in range(2):
                    cap = caps[e * 2 + par]
                    nc.gpsimd.dma_scatter_add(
                        outs[par][:],
                        xb[:, par * gpar:(par + 1) * gpar, :],
                        idx_sb[:, (e * 2 + par) * (pp // 16):
                               (e * 2 + par) * (pp // 16) + cap // 16],
                        cap,
                        cap,
                        OUT_DIM,
                        queue_num=par,
                    )

    nc.compile()
    return nc


def _get_program(pp, caps):
    key = (pp, caps)
    if key not in _cache:
        _cache[key] = _build_program(pp, caps)
    return _cache[key]


def _dedup_head(Yh, indh):
    """Pre-sum rows sharing a slot (exact: same W)."""
    order = np.argsort(indh, kind="stable")
    sind = indh[order]
    starts = np.concatenate(([0], np.nonzero(np.diff(sind))[0] + 1))
    sums = np.add.reduceat(Yh[order].astype(np.float32), starts, axis=0)
    return sums, sind[starts]


def kernel(Y, Ind, T, W):
    Y = np.asarray(Y, dtype=np.float32)
    Ind = np.asarray(Ind)
    W = np.asarray(W, dtype=np.float32)
    assert int(T) == T_SLOTS and Y.shape == (B, HEADS, K, HEAD_DIM)

    w_in = np.ascontiguousarray(
        W.transpose(1, 0, 2).reshape(HEAD_DIM, HEADS * OUT_DIM)
    ).astype(np.float16)

    # Host prep: dedup per head, split by slot parity.
    per_core = []
    cnts = np.zeros((B, HEADS * 2), dtype=np.int64)
    for b in range(B):
        heads = []
        for e in range(HEADS):
            sums, slots = _dedup_head(Y[b, e], Ind[b, e].astype(np.int64))
            parts = []
            for par in range(2):
                m = (slots % 2) == par
                parts.append((sums[m], (slots[m] // 2).astype(np.int16)))
                cnts[b, e * 2 + par] = int(m.sum())
            heads.append(parts)
        per_core.append(heads)
    maxcnt = int(cnts.max())
    pp = max(PP, -(-maxcnt // 128) * 128)
    kh = 2 * pp
    # Per-call num_idxs cap: shared across cores (SPMD), tight per call.
    caps = tuple(int(-(-int(c) // 16) * 16) or 16
                 for c in cnts.max(axis=0))

    in_maps = []
    for b in range(B):
        ytb = np.zeros((HEAD_DIM, HEADS * kh), dtype=np.float16)
        idx_cols = []
        for e in range(HEADS):
            for par in range(2):
                rows, slots = per_core[b][e][par]
                cnt = len(slots)
                base = e * kh + par * pp
                ytb[:, base:base + cnt] = rows.T.astype(np.float16)
                idxp = np.zeros(pp, dtype=np.int16)
                idxp[:cnt] = slots
                idx_cols.append(np.tile(idxp.reshape(pp // 16, 16).T, (8, 1)))
        idx_in = np.ascontiguousarray(np.concatenate(idx_cols, axis=1),
                                      dtype=np.int16)
        in_maps.append({"yt": ytb, "w": w_in, "idx": idx_in})

    nc = _get_program(pp, caps)

    # The first execution of a freshly compiled NEFF occasionally wedges a
    # core (NRT_EXEC_UNIT_UNRECOVERABLE); a retry on a fresh execute has
    # been observed to recover.
    last_exc = None
    for attempt in range(3):
        try:
            res = run_bass_kernel_spmd(
                nc, in_maps, core_ids=list(range(NCORES)),
                trace=os.environ.get("ES_TRACE", "0") == "1",
            )
            break
        except Exception as exc:  # noqa: BLE001 - device flake, retry
            last_exc = exc
            import time as _time
            _time.sleep(2.0)
    else:
        raise last_exc
    kernel.last_results = res
    out = np.empty((B, T_SLOTS, OUT_DIM), dtype=np.float32)
    for b in range(B):
        out[b, 0::2] = res.results[b]["out_even"].astype(np.float32)
        out[b, 1::2] = res.results[b]["out_odd"].astype(np.float32)
    return out
